# revision 49
# baseline (speedup 1.0000x reference)
"""Multi-head attention (B=2, S=2048, D=1024, H=16) on 8 TRN2 NeuronCores.

Sharding: tensor-parallel over heads x data-parallel over batch.
Core c handles batch b = c//4, head group g = c%4 (4 heads, 256 cols).
W_q/W_k/W_v are split column-wise per group, W_o row-wise; each core
produces a partial [S, D] output, reduced on the host (the W_o
contraction is a pure sum over head groups; b_v/b_o folded in on host).

Device kernel (per core), matmuls bf16/fp16 with fp32 PSUM accumulation:
  - K^T, Q^T projections in transposed layout [dk*2, S], V in natural
    layout [S, 4*(dk+1)] with a ones column per head (softmax
    denominator rows come free out of the ctx matmul).
  - logits are computed per HEAD-PAIR: the two heads of a QT/KT column
    live in partitions 0:64 and 64:128, so their two 64-contraction
    matmuls land on disjoint PE row-groups (tile_position (0,0)/(64,0))
    and execute concurrently -- 2x the 64-row matmul rate.
  - softmax exp runs entirely on the ACT engine (exact exp -> fp16,
    N=1024 per instruction); keeping the DVE nearly idle matters more
    than exp parallelism: the chip-wide power limiter lowers every
    clock when multiple engines run hot concurrently (measured +20%
    slowdown with a two-engine exp split).
  - ctx^T accumulated per head over k-chunks with ones-augmented V,
    normalized via DVE with a gpsimd partition-broadcast of the
    reciprocal denominators.
  - out partial = ctx^T-chunks @ W_o rows; last span is two-pass so
    only half the W_o work remains in the epilogue.
"""

import numpy as np
import ml_dtypes
from contextlib import ExitStack

import concourse.bass as bass
import concourse.tile as tile
from concourse import bacc, mybir
from concourse.bass_utils import run_bass_kernel_spmd

BF16 = mybir.dt.bfloat16
F16 = mybir.dt.float16
F32 = mybir.dt.float32
I16 = mybir.dt.int16

D = 1024            # model dim
H = 16              # heads
DK = 64             # head dim
NCORES = 8
GPB = 4             # head groups per batch (= cores per batch)
HPG = H // GPB      # 4 heads per core
HD = HPG * DK       # 256 cols per group
HAUG = DK + 1       # 65: head block width in augmented-V layout
SP = 512            # q-span / free-dim tile
SCALE = 1.0 / np.sqrt(DK)

# Schraudolph fast-exp constants (fp16 variant): exp(s*x) ~= bitcast_fp16(
# int16(x * SA + SB)).  SB calibrated for round-to-nearest conversion.
SA = float(SCALE * 1024.0 / np.log(2.0))
SB = float(15.0 * 1024.0 - 19.0)


def build(S):
    NQS = S // SP       # q spans
    NSC = S // 128      # sequence chunks (k side)
    NDC = D // 128      # model-dim chunks
    SI = SP // 128      # s-chunks per q-span
    NDH = D // SP       # W_o output column tiles
    JPS = 2 * NSC       # jobs per span: (pair m, k-chunk sc)
    TOT = NQS * JPS
    LAG = 2

    nc = bacc.Bacc("TRN2", target_bir_lowering=False, debug=False)
    HA = HPG * HAUG     # 260: augmented V width
    xT_e = nc.dram_tensor("xT", [S // SP, 128, NDC, SP], BF16, kind="ExternalInput")
    wq_e = nc.dram_tensor("wq", [128, NDC, HD], BF16, kind="ExternalInput")
    wk_e = nc.dram_tensor("wk", [128, NDC, HD], BF16, kind="ExternalInput")
    wv_e = nc.dram_tensor("wv", [128, NDC, HA], BF16, kind="ExternalInput")
    wo_e = nc.dram_tensor("wo", [128, 2, D], BF16, kind="ExternalInput")
    bq_e = nc.dram_tensor("bq", [128, 2], F32, kind="ExternalInput")
    bk_e = nc.dram_tensor("bk", [128, 2], F32, kind="ExternalInput")
    # fp16 partials (vs f32) halve the output DMA -- the host upcasts
    out_e = nc.dram_tensor("out", [S, D], F16, kind="ExternalOutput")
    # last-span W_o is two-pass; the early (m=0) half lands here and the
    # host adds it in (avoids an on-device merge on the critical tail)
    out2_e = nc.dram_tensor("out2", [SP, D], F16, kind="ExternalOutput")

    ADD = mybir.AluOpType.add
    MULT = mybir.AluOpType.mult
    EXP = mybir.ActivationFunctionType.Exp
    COPY = mybir.ActivationFunctionType.Copy

    with tile.TileContext(nc) as tc, ExitStack() as ctx:
        const = ctx.enter_context(tc.tile_pool(name="const", bufs=1))
        qpool = ctx.enter_context(tc.tile_pool(name="qpool", bufs=2))
        cpool = ctx.enter_context(tc.tile_pool(name="cpool", bufs=2))
        ptp = ctx.enter_context(tc.tile_pool(name="ptp", bufs=6))
        obp = ctx.enter_context(tc.tile_pool(name="obp", bufs=3))
        smp = ctx.enter_context(tc.tile_pool(name="smp", bufs=3))
        ps_lg = ctx.enter_context(tc.tile_pool(name="ps_lg", bufs=2, space="PSUM"))
        # one shared 4-slot ring for ctx pairs + all projection/W_o tiles:
        # pair handoffs (2 old cps + 2 new cps) never stall; interleaved
        # dense ops absorb the transient slot pressure instead
        ps_cx = ctx.enter_context(tc.tile_pool(name="ps_cx", bufs=4, space="PSUM"))
        ps_wo = ps_cx

        wq_sb = const.tile([128, NDC, HD], BF16, name="wq_sb")
        wk_sb = const.tile([128, NDC, HD], BF16, name="wk_sb")
        wv_sb = const.tile([128, NDC, HA], BF16, name="wv_sb")
        wo_sb = const.tile([128, 2, D], BF16, name="wo_sb")
        bq_sb = const.tile([128, 2], F32, name="bq_sb")
        bk_sb = const.tile([128, 2], F32, name="bk_sb")
        warm_sb = const.tile([1, 16], F32, name="warm_sb")
        xT_sb = [const.tile([128, NDC, SP], BF16, name=f"xT{q}") for q in range(NQS)]
        KT_sb = const.tile([128, 2, S], BF16, name="KT_sb")
        V_sb = const.tile([128, NSC, HA], BF16, name="V_sb")

        # input DMAs on one queue (the two hwdge queues share bandwidth),
        # ordered to exactly match prologue/fill consumption; xT spans
        # split in half so consumers gate on finer chunks.
        nc.sync.dma_start(wk_sb[:, :, 0:128], wk_e.ap()[:, :, 0:128])
        nc.sync.dma_start(bk_sb[:], bk_e.ap())
        nc.sync.dma_start(xT_sb[0][:, 0:2, :], xT_e.ap()[0, :, 0:2, :])
        nc.sync.dma_start(xT_sb[0][:, 2:4, :], xT_e.ap()[0, :, 2:4, :])
        nc.sync.dma_start(xT_sb[0][:, 4:, :], xT_e.ap()[0, :, 4:, :])
        nc.sync.dma_start(wv_sb[:], wv_e.ap())
        nc.sync.dma_start(wq_sb[:], wq_e.ap())
        nc.sync.dma_start(bq_sb[:], bq_e.ap())
        nc.sync.dma_start(wk_sb[:, :, 128:], wk_e.ap()[:, :, 128:])
        for q in range(1, NQS):
            nc.sync.dma_start(xT_sb[q][:, 0:4, :], xT_e.ap()[q, :, 0:4, :])
            nc.sync.dma_start(xT_sb[q][:, 4:, :], xT_e.ap()[q, :, 4:, :])
        nc.sync.dma_start(wo_sb[:], wo_e.ap())

        # preload the exp table set during the initial DMA wait
        nc.vector.memset(warm_sb[:], 0.0)
        warm_pt = smp.tile([1, 16], BF16, name="warm_pt")
        nc.scalar.activation(warm_pt[:], warm_sb[:], EXP)

        # all exps on ACT: keeping the DVE cool raises the chip-wide
        # power-limited clocks (measured 241us -> 209us vs a 9:7 split
        # in the same thermal state); exact exp also halves the error
        def exp_on_dve(g):
            return False

        # copies alternate engines strictly, independent of exp parity
        _cp = [0]

        def free_copy(g, out, in_):
            nc.vector.tensor_copy(out, in_)

        def free_add(g, out, in_, bias):
            # ACT Copy can't take an AP bias -- these are rare, keep on DVE
            nc.vector.tensor_scalar(out, in_, bias, None, ADD)

        # ---- projection groups (each a complete psum-tile lifetime) ----
        # prologue/span-0 fills borrow the (then idle) ctx psum slots so
        # they triple-buffer; steady-state interleave ops use ps_wo.
        def emit_kproj_group(m, q, g=None, pool=None):
            p = pool or ps_wo
            ps = p.tile([128, SP], F32, tag="ctx" if p is ps_cx else "wo",
                        name="kps")
            for c in range(NDC):
                nc.tensor.matmul(
                    ps[:], wk_sb[:, c, m * 128:(m + 1) * 128],
                    xT_sb[q][:, c, :],
                    start=(c == 0), stop=(c == NDC - 1))
            free_add(g, KT_sb[:, m, q * SP:(q + 1) * SP], ps[:],
                     bk_sb[:, m:m + 1])

        def emit_qproj_group(QTn, qsrc, m, g=None, pool=None):
            p = pool or ps_wo
            ps = p.tile([128, SP], F32, tag="ctx" if p is ps_cx else "wo",
                        name="qps")
            for c in range(NDC):
                nc.tensor.matmul(
                    ps[:], wq_sb[:, c, m * 128:(m + 1) * 128],
                    xT_sb[qsrc][:, c, :],
                    start=(c == 0), stop=(c == NDC - 1))
            free_add(g, QTn[:, m, :], ps[:], bq_sb[:, m:m + 1])

        def emit_vproj_group(sc, g=None, pool=None):
            q, si = divmod(sc, SI)
            p = pool or ps_wo
            ps = p.tile([128, HA], F32, tag="ctx" if p is ps_cx else "wo",
                        name="vps")
            for c in range(NDC):
                nc.tensor.matmul(
                    ps[:], xT_sb[q][:, c, si * 128:(si + 1) * 128],
                    wv_sb[:, c, :],
                    start=(c == 0), stop=(c == NDC - 1))
            free_copy(g, V_sb[:, sc, :], ps[:])
            # ones columns for the softmax-denominator rows
            vsc = V_sb[:, sc, :].rearrange("p (h x) -> p h x", x=HAUG)
            nc.vector.memset(vsc[:, :, DK:DK + 1], 1.0)

        # ---- attention machinery -------------------------------------
        def emit_lg_exp(QT, m, sc, g):
            lg = ps_lg.tile([128, 2 * SP], F32, tag="lg", name="lg")
            for i, r in enumerate((0, 64)):
                nc.tensor.matmul(
                    lg[:, i * SP:(i + 1) * SP],
                    KT_sb[r:r + 64, m, sc * 128:(sc + 1) * 128],
                    QT[r:r + 64, m, :],
                    start=True, stop=True)
            if not exp_on_dve(g):
                pt = ptp.tile([128, 2 * SP], F16, name="pt")
                nc.scalar.activation(pt[:], lg[:], EXP, scale=float(SCALE))
                return pt[:]
            pt = ptp.tile([128, 2 * SP], I16, name="pt")
            nc.vector.tensor_scalar(pt[:], lg[:], SA, SB, MULT, ADD)
            return pt[:].bitcast(F16)

        def emit_ctx(CT, cps2, m, sc, ptv):
            if sc == 0:
                cps2[m] = (ps_cx.tile([HAUG, SP], F32, tag="ctx", name="cpsA"),
                           ps_cx.tile([HAUG, SP], F32, tag="ctx", name="cpsB"))
            for i in range(2):
                h = 2 * m + i
                nc.tensor.matmul(
                    cps2[m][i][:], V_sb[:, sc, h * HAUG:(h + 1) * HAUG],
                    ptv[:, i * SP:(i + 1) * SP],
                    start=(sc == 0), stop=(sc == NSC - 1))
            if sc == NSC - 1:
                return [make_norm(CT, 2 * m + i, cps2[m][i]) for i in range(2)]
            return None

        def make_norm(CT, h, cps):
            # deferred one job so nothing here waits at a queue head;
            # the partition broadcast runs on the (otherwise idle) gpsimd
            def run():
                m, r = divmod(h, 2)
                r *= 64
                # denominator row lifted out of PSUM on ACT (keeps the DVE
                # queue free for exp conversions)
                sm = smp.tile([1, SP], F32, name="sm")
                nc.vector.tensor_copy(sm[:], cps[DK:DK + 1, :])
                rc = smp.tile([1, SP], F32, name="rc")
                nc.vector.reciprocal_approx_fast(rc[:], sm[:])
                bc = smp.tile([64, SP], F32, name="bc")
                nc.gpsimd.partition_broadcast(bc[:], rc[:])
                nc.vector.tensor_tensor(
                    CT[r:r + 64, m, :], cps[0:DK, :], bc[:], MULT)
            return run

        def make_wo_op(q, si, CT):
            # both dh halves -> one [128, 1024] fp16 tile -> one 2KB-line DMA
            def run(g, pool):
                sc = SI * q + si
                ob = obp.tile([128, 2 * SP], F16, name="ob")
                for dh in range(NDH):
                    po = ps_wo.tile([128, SP], F32, tag="ctx", name="po")
                    nc.tensor.matmul(
                        po[:], CT[:, 0, si * 128:(si + 1) * 128],
                        wo_sb[:, 0, dh * SP:(dh + 1) * SP],
                        start=True, stop=False)
                    nc.tensor.matmul(
                        po[:], CT[:, 1, si * 128:(si + 1) * 128],
                        wo_sb[:, 1, dh * SP:(dh + 1) * SP],
                        start=False, stop=True)
                    free_copy(g + dh, ob[:, dh * SP:(dh + 1) * SP], po[:])
                nc.sync.dma_start(
                    out_e.ap()[sc * 128:(sc + 1) * 128, :], ob[:])
            return run

        def make_passa_op(si, CT):
            # last-span m=0 W_o half -> out2 (host adds the two halves)
            def run(g, pool):
                ob = obp.tile([128, 2 * SP], F16, name="ob")
                for dh in range(NDH):
                    po = ps_wo.tile([128, SP], F32, tag="ctx", name="po")
                    nc.tensor.matmul(
                        po[:], CT[:, 0, si * 128:(si + 1) * 128],
                        wo_sb[:, 0, dh * SP:(dh + 1) * SP],
                        start=True, stop=True)
                    free_copy(g + dh, ob[:, dh * SP:(dh + 1) * SP], po[:])
                nc.sync.dma_start(
                    out2_e.ap()[si * 128:(si + 1) * 128, :], ob[:])
            return run

        # ---- prologue: only what job 0 needs -- K^T m=0 span q=0,
        # V 0..3, Q^T span0 m=0.  Everything else streams in as fills.
        emit_kproj_group(0, 0, pool=ps_cx)
        for sc in range(SI):
            emit_vproj_group(sc, pool=ps_cx)
        QT_t = {0: qpool.tile([128, 2, SP], BF16, name="QT")}
        emit_qproj_group(QT_t[0], 0, 0, pool=ps_cx)

        # ---- one flat software pipeline over all (span, pair, chunk) --
        # work queue: (min_job, closure(g, pool)); span 0 pops two per
        # job (on alternating psum rings) to drain the big fill backlog
        def kp(m, q):
            return lambda g, pool: emit_kproj_group(m, q, g, pool=pool)

        def vp(sc):
            return lambda g, pool: emit_vproj_group(sc, g, pool=pool)

        def qp(qt, qsrc, m):
            return lambda g, pool: emit_qproj_group(QT_t[qt], qsrc, m, g,
                                                    pool=pool)

        wq = [(0, f) for f in [
            qp(0, 0, 1), kp(1, 0), kp(0, 1), vp(4), vp(5), vp(6), vp(7),
            kp(1, 1), kp(0, 2), vp(8), vp(9), vp(10), vp(11), kp(0, 3),
            vp(12), vp(13), vp(14), vp(15), kp(1, 2), kp(1, 3),
        ]]

        pend = []          # (emit_job, q, m, sc, ptv)
        norms_due = []     # (due_job, closure)
        CT_t = {}
        cps_t = {}
        for g in range(TOT):
            q, jidx = divmod(g, JPS)
            m, sc = divmod(jidx, NSC)
            if jidx == 0:
                CT_t[q] = cpool.tile([128, 2, SP], BF16, name="CT")
                cps_t[q] = {}
                if q + 1 < NQS:
                    QT_t[q + 1] = qpool.tile([128, 2, SP], BF16, name="QT")
                    wq.append((g, qp(q + 1, q + 1, 0)))
                    wq.append((g, qp(q + 1, q + 1, 1)))
                if q > 0:
                    # gate past the previous pair's staggered B-norm (due
                    # span_start+4) so the first W_o matmul never reads
                    # not-yet-normalized CT rows
                    for si in range(SI):
                        wq.append((g + 4, make_wo_op(q - 1, si, CT_t[q - 1])))
                if q == NQS - 1:
                    wq.extend((g + 20, make_passa_op(si, CT_t[q]))
                              for si in range(SI))

            # 1) due norms first (DVE/gpsimd only; no PE content)
            while norms_due and norms_due[0][0] <= g:
                norms_due.pop(0)[1]()
            # 2) this job's logits pair + exp
            ptv = emit_lg_exp(QT_t[q], m, sc, g)
            pend.append((g, q, m, sc, ptv))
            # 3) lagged ctx pops (new-pair head entries wait one extra job)
            while pend:
                e, eq, em, esc, eptv = pend[0]
                due = e + (LAG + 1 if esc == 0 else LAG)
                if due > g:
                    break
                pend.pop(0)
                nrm = emit_ctx(CT_t[eq], cps_t[eq], em, esc, eptv)
                if nrm is not None:
                    norms_due.append((g + 1, nrm[0]))
                    norms_due.append((g + 2, nrm[1]))
            # 4) interleaved dense-work pops (two per job while the span-0
            # fill backlog lasts, on alternating psum rings)
            for k in range(2 if g < JPS else 1):
                if wq and wq[0][0] <= g:
                    pool = ps_cx if (g < JPS and k == 0) else ps_wo
                    wq.pop(0)[1](g, pool)

        # ---- epilogue: drain pipeline, finish last-span W_o ----------
        g = TOT
        while pend or norms_due or wq:
            while norms_due and norms_due[0][0] <= g:
                norms_due.pop(0)[1]()
            if pend:
                e, eq, em, esc, eptv = pend.pop(0)
                nrm = emit_ctx(CT_t[eq], cps_t[eq], em, esc, eptv)
                if nrm is not None:
                    norms_due.append((g + 1, nrm[0]))
                    norms_due.append((g + 2, nrm[1]))
            elif wq:
                wq.pop(0)[1](g, ps_wo)
            g += 1

        # keep the PE busy through the final norm chains so the HAM clock
        # gate stays at full rate for the last W_o pass (results unused):
        # three free-running matmuls bridge the first half of the norm
        # window, two more gated on the first head's normalized CT rows
        # bridge the rest.
        CTl = CT_t[NQS - 1]
        warm_ps = ps_cx.tile([128, SP], F32, tag="ctx", name="warm_ps")
        for i in range(3):
            nc.tensor.matmul(
                warm_ps[:], wo_sb[:, 0, 0:128], wo_sb[:, 1, 0:SP],
                start=(i == 0), stop=False)
        for i in range(2):
            nc.tensor.matmul(
                warm_ps[:], CTl[0:64, 1, 0:128], wo_sb[0:64, 1, 0:SP],
                start=False, stop=(i == 1))
        for si in range(SI):
            sc = SI * (NQS - 1) + si
            ob = obp.tile([128, 2 * SP], F16, name="ob")
            for dh in range(NDH):
                po = ps_cx.tile([128, SP], F32, tag="ctx", name="po")
                nc.tensor.matmul(
                    po[:], CTl[:, 1, si * 128:(si + 1) * 128],
                    wo_sb[:, 1, dh * SP:(dh + 1) * SP],
                    start=True, stop=True)
                free_copy(si + dh, ob[:, dh * SP:(dh + 1) * SP], po[:])
            nc.sync.dma_start(out_e.ap()[sc * 128:(sc + 1) * 128, :], ob[:])

    nc.compile()
    return nc


_NC_CACHE = {}


def get_nc(S):
    if S not in _NC_CACHE:
        _NC_CACHE[S] = build(S)
    return _NC_CACHE[S]


def make_in_maps(x, W_q, b_q, W_k, b_k, W_v, b_v, W_o, b_o):
    B, S, _ = x.shape
    bf = ml_dtypes.bfloat16
    in_maps = []
    for core in range(NCORES):
        b, g = divmod(core, GPB)
        sl = slice(HD * g, HD * (g + 1))
        wv_aug = np.zeros((D, HPG * HAUG), np.float32)
        wv_block = np.asarray(W_v[:, sl]).reshape(D, HPG, DK)
        wv_aug.reshape(D, HPG, HAUG)[:, :, :DK] = wv_block
        def wtile(w):
            # [D, N] -> [128, D//128, N] partition-major chunk layout
            return np.ascontiguousarray(
                np.asarray(w).reshape(D // 128, 128, -1).transpose(1, 0, 2))
        in_maps.append({
            "xT": np.ascontiguousarray(
                np.asarray(x[b]).T.reshape(D // 128, 128, S // SP, SP)
                .transpose(2, 1, 0, 3)).astype(bf),
            "wq": wtile(W_q[:, sl]).astype(bf),
            "wk": wtile(W_k[:, sl]).astype(bf),
            "wv": wtile(wv_aug).astype(bf),
            "wo": np.ascontiguousarray(
                np.asarray(W_o[sl, :]).reshape(2, 128, D)
                .transpose(1, 0, 2)).astype(bf),
            "bq": np.ascontiguousarray(
                np.asarray(b_q[sl]).reshape(2, 128).T).astype(np.float32),
            "bk": np.ascontiguousarray(
                np.asarray(b_k[sl]).reshape(2, 128).T).astype(np.float32),
        })
    return in_maps


def unshard(results, x, W_o, b_v, b_o):
    B, S, _ = x.shape
    out = np.zeros((B, S, D), np.float32)
    for core in range(NCORES):
        b = core // GPB
        out[b] += results[core]["out"].astype(np.float32)
        out[b, S - SP:, :] += results[core]["out2"].astype(np.float32)
    const = np.asarray(b_v).astype(np.float64) @ np.asarray(W_o).astype(np.float64)
    const += np.asarray(b_o).astype(np.float64)
    out += const.astype(np.float32)[None, None, :]
    return out


def run(inputs, trace=False):
    x = np.asarray(inputs["x"])
    nc = get_nc(x.shape[1])
    in_maps = make_in_maps(
        x, inputs["W_q"], inputs["b_q"], inputs["W_k"], inputs["b_k"],
        inputs["W_v"], inputs["b_v"], inputs["W_o"], inputs["b_o"])
    def attempt():
        res = run_bass_kernel_spmd(
            nc, in_maps, core_ids=list(range(NCORES)), trace=trace)
        # force materialization here: PJRT surfaces device errors lazily
        res.results = [{k: np.asarray(v) for k, v in r.items()}
                       for r in res.results]
        return res
    try:
        res = attempt()
    except Exception:
        # transient device errors (e.g. NRT_EXEC_UNIT_UNRECOVERABLE) clear
        # on re-execution of the same NEFF
        res = attempt()
    out = unshard(res.results, x, inputs["W_o"], inputs["b_v"], inputs["b_o"])
    return out, res


def kernel(**inputs):
    out, _ = run(inputs, trace=False)
    return out
